# revision 33
# baseline (speedup 1.0000x reference)
"""Neural ODE (explicit Euler, 20 steps) Trainium2 Bass kernel, v2.

z_{s+1} = z_s + h * (tanh(z_s @ W1 + b1) @ W2 + b2),  z0: [8192, 512] f32.

Key algebraic restructuring (exact): track the pre-activation state
x_s = z_s @ W1 (+b1 via bias path) instead of z_s.  With
M = h * W2 @ W1 (host-precomputed):

    a_s     = tanh(x_s / S)            (x held scaled by S for fp8 range)
    x_{s+1} = x_s + a_s @ (S*M)        <- the ONLY matmul per step
    z_final = z0 + h * (sum_s a_s) @ W2  (+ 20*h*b2)

This halves the per-step matmul work vs the naive two-matmul loop, and the
per-step matmul runs in fp8e4m3 with perf_mode=DoubleRow (2 fp8 weights per
PE cell -> 2x throughput; virtual 128x256 array).  The accumulated state x
lives in PSUM across all 20 steps (matmuls accumulate in place with
start=False), read out each step by the ACT engine's tanh which writes the
fp8 activation planes directly.  A = sum_s a_s is accumulated on the DVE in
fp16 via step-pair sums: p_k = a_{2k}+a_{2k+1} (fp8 sources -> 1x DVE rate,
unavoidable) then A += p_k in all-fp16 (2x_1P rate) -- 68us total DVE vs
92us for the naive per-step fp32 accumulation, keeping DVE under the ACT
bound.  Init (z0@W1) and final (A@W2)
matmuls run in fp16 so the displacement z_final - z0 only carries fp8 noise
through the dynamics, not through the final projection (numpy model of this
scheme: rel err ~7e-3 vs the fp32 reference; gate is 2e-2).

Data parallel over 8 cores (1024 batch rows each), feature-major layout,
batch split into 2 chunks of 512 columns so the per-chunk pipeline
PE(mm) -> ACT(tanh) -> PE(next mm) overlaps across chunks.  Steady state is
ACT-bound at ~3.95us/step (2 tanh ops of [128,2048] @ 1.2GHz); PE is at
~3.44us/step (16 DoubleRow matmuls @ 215ns).

Head: input DMAs for z16/w1s are split per contraction-plane so the init
matmuls start as soon as the first 256KB lands, and the PE warmup bridges
the DMA latency so HAM never re-throttles.  Tail: output DMAs alternate
between two engine queues.
"""

import numpy as np
import ml_dtypes

P = 128
D = 512
B_FULL = 8192
NCORES = 8
BSH = B_FULL // NCORES  # 1024 batch rows per core
NSTEPS = 20
CB = 512                # batch columns per chunk
NCHUNK = BSH // CB      # 2 chunks
FT = D // P             # 4 feature tiles
S = 128.0               # fp32 state scale (keeps fp8 M~ in normal range)
NWARM = 26              # PE prewarm matmuls (HAM clock ramp) during DMA

_CACHE = {}


def _build_nc(has_bias):
    import concourse.bacc as bacc
    import concourse.mybir as mybir
    import concourse.tile as tile
    from concourse.masks import make_identity

    f32 = mybir.dt.float32
    f16 = mybir.dt.float16
    f8 = mybir.dt.float8e4
    Tanh = mybir.ActivationFunctionType.Tanh
    DR = mybir.MatmulPerfMode.DoubleRow

    nc = bacc.Bacc("TRN2", target_bir_lowering=False, debug=False)
    # All inputs arrive pre-tiled in SBUF layout (4KB contiguous rows) so
    # each tensor is one large DMA at near-peak HBM bandwidth.  The init-
    # critical pair (z0 chunk-0 + S*W1) ships as two 512KB halves so the
    # first half unblocks init waves kt=0,1 ~2us earlier.
    # head_a planes: [z0k0, z0k1, w1k0, w1k1]; head_b: [z0k2, z0k3, w1k2, w1k3]
    ha_in = nc.dram_tensor("head_a", [P, FT, D], f16, kind="ExternalInput")
    hb_in = nc.dram_tensor("head_b", [P, FT, D], f16, kind="ExternalInput")
    z1_in = nc.dram_tensor("z16c1", [P, FT, CB], f16, kind="ExternalInput")
    w2h_in = nc.dram_tensor("w2h", [P, FT, D], f16, kind="ExternalInput")  # h*W2
    mq_in = nc.dram_tensor("mq", [P, FT, D], f8, kind="ExternalInput")  # S*h*W2@W1
    if has_bias:
        # biases[p, jt, s] = b1[jt*128+p] + s * (b2 @ W1 * h)[jt*128+p]
        b_in = nc.dram_tensor("biases", [P, FT, NSTEPS], f32, kind="ExternalInput")
        # bfin_row[0, j] = NSTEPS * h * b2[j]
        bf_in = nc.dram_tensor("bfin", [1, D], f16, kind="ExternalInput")
    z_out = nc.dram_tensor("zout", [D, BSH], f16, kind="ExternalOutput")

    zout_t = z_out.ap().rearrange("(ft p) b -> p ft b", p=P)

    def cs(c):
        return slice(c * CB, (c + 1) * CB)

    with tile.TileContext(nc) as tc:
        with (
            tc.tile_pool(name="wpool", bufs=1) as wpool,
            tc.tile_pool(name="a8pool", bufs=4) as a8pool,
            tc.tile_pool(name="ppool", bufs=3) as ppool,
            tc.tile_pool(name="apool", bufs=3) as apool,
            tc.tile_pool(name="opool", bufs=1) as opool,
            tc.tile_pool(name="ps", bufs=1, space="PSUM") as ps,
        ):
            # persistent PSUM state: x~ = S * (z_s @ W1), one 4-bank tile per
            # chunk; bank (c, jt) = xps[c][:, jt, :]
            xps = [ps.tile([P, FT, CB], f32, tag=f"x{c}", name=f"x{c}")
                   for c in range(NCHUNK)]

            # ---- identity + ACT tanh-table preload ----
            ident = wpool.tile([P, P], f32, tag="id")
            make_identity(nc, ident[:])
            warm_sink = wpool.tile([P, P], f32, tag="warm")
            nc.scalar.activation(
                warm_sink[0:1, 0:1], ident[0:1, 0:1], Tanh, scale=1.0,
            )
            ident16 = wpool.tile([P, P], f16, tag="id16")
            nc.vector.tensor_copy(ident16[:], ident[:])

            ha_sb = wpool.tile([P, FT, D], f16, tag="head_a")
            hb_sb = wpool.tile([P, FT, D], f16, tag="head_b")
            nc.sync.dma_start(ha_sb[:], ha_in.ap())
            nc.sync.dma_start(hb_sb[:], hb_in.ap())
            z1_sb = wpool.tile([P, FT, CB], f16, tag="z16_1", name="z16_1")
            mq_sb = wpool.tile([P, FT, D], f8, tag="mq")
            nc.sync.dma_start(z1_sb[:], z1_in.ap())
            nc.sync.dma_start(mq_sb[:], mq_in.ap())

            def z0sl(c, kt):
                # z0 chunk c, feature plane kt
                if c == 1:
                    return z1_sb[:, kt, :]
                return ha_sb[:, kt, :] if kt < 2 else hb_sb[:, kt - 2, :]

            def w1sl(kt, jt):
                sl = slice(jt * P, (jt + 1) * P)
                return (ha_sb[:, 2 + kt, sl] if kt < 2
                        else hb_sb[:, kt, sl])
            if has_bias:
                bias_sb = wpool.tile([P, FT, NSTEPS], f32, tag="bias")
                nc.sync.dma_start(bias_sb[:], b_in.ap())

            # ---- PE prewarm with junk fp16 matmuls into the x banks; the
            # init matmuls below overwrite with start=True.  Bridges the
            # ~8us z16 DMA latency so HAM stays at full clock. ----
            for i in range(NWARM):
                nc.tensor.matmul(
                    xps[i % 2][:, (i // 2) % FT, 0:P], ident16[:], ident16[:],
                    start=True, stop=True, skip_group_check=True,
                )
            nc.vector.tensor_copy(
                warm_sink[:], xps[(NWARM - 1) % 2][:, ((NWARM - 1) // 2) % FT, 0:P])

            # ---- init: x~_0 = z0 @ (S*W1), fp16, kt-outer so each 256KB
            # DMA piece unblocks 8 matmuls ----
            for c in range(NCHUNK):
                for kt in range(FT):
                    for jt in range(FT):
                        nc.tensor.matmul(
                            xps[c][:, jt, :],
                            w1sl(kt, jt),
                            z0sl(c, kt),
                            start=(kt == 0), stop=(kt == FT - 1),
                            skip_group_check=True,
                        )

            # needed only at the final projection; issued after the critical
            # head pieces
            w2h_sb = wpool.tile([P, FT, D], f16, tag="w2h")
            nc.sync.dma_start(w2h_sb[:], w2h_in.ap())
            if has_bias:
                bfin_sb = wpool.tile([1, D], f16, tag="bfin")
                nc.sync.dma_start(bfin_sb[:], bf_in.ap())
                ones_sb = wpool.tile([1, CB], f16, tag="ones")
                nc.vector.memset(ones_sb[:], 1.0)

            # ---- 20 steps: (x~ += a8 @ M~ for s>0), a8 = tanh(x~/S);
            # DVE accumulates A in fp16 from step-pair sums.
            a8 = [None] * NCHUNK       # current a8
            a8p = [None] * NCHUNK      # previous step's a8 (for pair sums)
            acc = [None] * NCHUNK      # fp16 running sum of pairs
            accp = [[None] * FT, [None] * FT]  # final per-plane A16 tiles
            for s in range(NSTEPS):
                for c in range(NCHUNK):
                    if s > 0:
                        for jt in range(FT):
                            for t in range(2):
                                nc.tensor.matmul(
                                    xps[c][:, jt, :],
                                    mq_sb[:, 2 * t:2 * t + 2, jt * P:(jt + 1) * P],
                                    a8[c][:, 2 * t:2 * t + 2, :],
                                    start=False, stop=(t == 1),
                                    perf_mode=DR,
                                    skip_group_check=True,
                                )
                    a8n = a8pool.tile([P, FT, CB], f8, tag=f"a8_{c}", name=f"a8_{c}_{s}")
                    if has_bias:
                        for jt in range(FT):
                            nc.scalar.activation(
                                a8n[:, jt, :], xps[c][:, jt, :], Tanh,
                                bias=bias_sb[:, jt, s:s + 1], scale=1.0 / S,
                            )
                    else:
                        nc.scalar.activation(
                            a8n[:], xps[c][:], Tanh, scale=1.0 / S,
                        )
                    a8p[c] = a8[c]
                    a8[c] = a8n
                    if s == NSTEPS - 1:
                        # last pair, whole tile (fewer DVE ops = fewer sem
                        # hops); shared bufs=1 buffer forces the DVE order
                        # p(c0) -> A16(c0) -> p(c1) -> A16(c1) so chunk 0's
                        # final matmuls unblock as early as possible
                        pl = ppool.tile([P, FT, CB], f16, tag="p_last",
                                        bufs=1, name=f"pl_{c}")
                        nc.vector.tensor_add(pl[:], a8p[c][:], a8n[:])
                        an = apool.tile([P, FT, CB], f16, tag=f"A16_{c}",
                                        name=f"A16_{c}")
                        nc.vector.tensor_add(an[:], acc[c][:], pl[:])
                        acc[c] = an
                    elif s % 2 == 1:
                        # pair sum p = a_{s-1} + a_s (fp8+fp8 -> fp16, 1x)
                        pn = ppool.tile([P, FT, CB], f16, tag=f"p_{c}",
                                        name=f"p_{c}_{s}")
                        nc.vector.tensor_add(pn[:], a8p[c][:], a8n[:])
                        if s == 1:
                            acc[c] = pn
                        else:
                            # A += p, all-fp16 (2x_1P)
                            an = apool.tile([P, FT, CB], f16, tag=f"A_{c}",
                                            name=f"A_{c}_{s}")
                            nc.vector.tensor_add(an[:], acc[c][:], pn[:])
                            acc[c] = an

            # ---- junk matmuls to keep HAM warm while the last A16 adds run
            # (chunk-0 banks only: chunk-1 targets would stall the PE queue
            # on the last tanh)
            for i in range(20):
                nc.tensor.matmul(
                    xps[0][:, i % FT, 0:P], ident16[:], ident16[:],
                    start=True, stop=True, skip_group_check=True,
                )

            # ---- final: z = z0 + (A fp16) @ (h*W2) (+ NSTEPS*h*b2) ----
            # All matmuls first, then all zo adds: interleaving them chains
            # whole-tile WAR deps (zo reads xps) and serializes the phase.
            for c in range(NCHUNK):
                for kt in range(FT):
                    for jt2 in range(FT):
                        nc.tensor.matmul(
                            xps[c][:, jt2, :],
                            w2h_sb[:, kt, jt2 * P:(jt2 + 1) * P],
                            acc[c][:, kt, :],
                            start=(kt == 0),
                            stop=(kt == FT - 1) and not has_bias,
                            skip_group_check=True,
                        )
                if has_bias:
                    for jt2 in range(FT):
                        nc.tensor.matmul(
                            xps[c][:, jt2, :],
                            bfin_sb[:, jt2 * P:(jt2 + 1) * P],
                            ones_sb[:],
                            start=False, stop=True,
                            skip_group_check=True,
                        )
                # z0 is re-read from the resident fp16 init input; output
                # ships fp16 (host casts back).  Chunk 1's four adds merge
                # into one DVE op (z1_sb is one contiguous tile) — its zo
                # chain is the exposed end of the kernel.
                if c == 1:
                    zo1 = opool.tile([P, FT, CB], f16, tag="zo_1", name="zo_1")
                    nc.vector.tensor_add(zo1[:], xps[1][:], z1_sb[:])
                    for jt2 in range(FT):
                        eng = nc.sync if jt2 % 2 == 0 else nc.scalar
                        eng.dma_start(zout_t[:, jt2, cs(1)], zo1[:, jt2, :])
                else:
                    for jt2 in range(FT):
                        zo = opool.tile([P, CB], f16, tag=f"zo_0_{jt2}",
                                        name=f"zo_0_{jt2}")
                        nc.vector.tensor_add(
                            zo[:], xps[0][:, jt2, :], z0sl(0, jt2),
                        )
                        eng = nc.sync if jt2 % 2 == 0 else nc.scalar
                        eng.dma_start(zout_t[:, jt2, cs(0)], zo[:])

    nc.finalize()
    return nc


def _get_nc(has_bias):
    key = ("nc", has_bias)
    if key not in _CACHE:
        _CACHE[key] = _build_nc(has_bias)
    return _CACHE[key]


def _prepare_inputs(z0, t, W1, b1, W2, b2):
    z0 = np.asarray(z0, dtype=np.float32)
    t = np.asarray(t, dtype=np.float32)
    W1 = np.asarray(W1, dtype=np.float32)
    b1 = np.asarray(b1, dtype=np.float64)
    W2 = np.asarray(W2, dtype=np.float32)
    b2 = np.asarray(b2, dtype=np.float64)

    h = (float(t[1]) - float(t[0])) / NSTEPS
    has_bias = bool(np.any(b1 != 0.0) or np.any(b2 != 0.0))

    zT16 = np.ascontiguousarray(z0.T).astype(np.float16)  # [D, B_FULL]
    # pre-tile to SBUF layout [p, plane, col] (plane q holds rows 128q+p):
    # each DMA then moves 4KB contiguous rows at near-peak HBM bandwidth
    zt = zT16.reshape(FT, P, B_FULL).transpose(1, 0, 2)  # [P, FT, B_FULL]

    def tile_w(w):  # [D, D] -> [P, FT, D]
        return np.ascontiguousarray(w.reshape(FT, P, D).transpose(1, 0, 2))

    w1s = tile_w((W1.astype(np.float64) * S).astype(np.float32).astype(np.float16))
    w2h = tile_w((W2.astype(np.float64) * h).astype(np.float32).astype(np.float16))
    M = (W2.astype(np.float64) @ W1.astype(np.float64)) * (h * S)
    mq = tile_w(M.astype(np.float32).astype(ml_dtypes.float8_e4m3))

    extras = {}
    if has_bias:
        wtb = (b2 @ W1.astype(np.float64)) * h      # [D]
        biases = np.stack(
            [b1 + s * wtb for s in range(NSTEPS)], axis=0
        ).astype(np.float32)                         # [NSTEPS, D]
        extras["biases"] = np.ascontiguousarray(
            biases.reshape(NSTEPS, FT, P).transpose(2, 1, 0))
        extras["bfin"] = np.ascontiguousarray(
            (NSTEPS * h * b2).astype(np.float32).astype(np.float16).reshape(1, D))

    in_maps = []
    for i in range(NCORES):
        z0c0 = zt[:, :, i * BSH:i * BSH + CB]          # [P, FT, CB]
        z0c1 = zt[:, :, i * BSH + CB:(i + 1) * BSH]
        m = {
            "head_a": np.ascontiguousarray(
                np.concatenate([z0c0[:, 0:2], w1s[:, 0:2]], axis=1)),
            "head_b": np.ascontiguousarray(
                np.concatenate([z0c0[:, 2:4], w1s[:, 2:4]], axis=1)),
            "z16c1": np.ascontiguousarray(z0c1),
            "w2h": w2h,
            "mq": mq,
        }
        m.update(extras)
        in_maps.append(m)
    return in_maps, has_bias


def _run(in_maps, has_bias, trace=False):
    from concourse import bass_utils

    nc = _get_nc(has_bias)
    res = bass_utils.run_bass_kernel_spmd(
        nc, in_maps, core_ids=list(range(NCORES)), trace=trace,
    )
    return res


def kernel(z0, t, W1, b1, W2, b2):
    in_maps, has_bias = _prepare_inputs(z0, t, W1, b1, W2, b2)
    res = _run(in_maps, has_bias)
    outT = np.concatenate([r["zout"] for r in res.results], axis=1)  # [D, B] f16
    return np.ascontiguousarray(outT.T).astype(np.float32)


# revision 34
# speedup vs baseline: 1.0056x; 1.0056x over previous
"""Neural ODE (explicit Euler, 20 steps) Trainium2 Bass kernel, v2.

z_{s+1} = z_s + h * (tanh(z_s @ W1 + b1) @ W2 + b2),  z0: [8192, 512] f32.

Key algebraic restructuring (exact): track the pre-activation state
x_s = z_s @ W1 (+b1 via bias path) instead of z_s.  With
M = h * W2 @ W1 (host-precomputed):

    a_s     = tanh(x_s / S)            (x held scaled by S for fp8 range)
    x_{s+1} = x_s + a_s @ (S*M)        <- the ONLY matmul per step
    z_final = z0 + h * (sum_s a_s) @ W2  (+ 20*h*b2)

This halves the per-step matmul work vs the naive two-matmul loop, and the
per-step matmul runs in fp8e4m3 with perf_mode=DoubleRow (2 fp8 weights per
PE cell -> 2x throughput; virtual 128x256 array).  The accumulated state x
lives in PSUM across all 20 steps (matmuls accumulate in place with
start=False), read out each step by the ACT engine's tanh which writes the
fp8 activation planes directly.  A = sum_s a_s is accumulated on the DVE in
fp16 via step-pair sums: p_k = a_{2k}+a_{2k+1} (fp8 sources -> 1x DVE rate,
unavoidable) then A += p_k in all-fp16 (2x_1P rate) -- 68us total DVE vs
92us for the naive per-step fp32 accumulation, keeping DVE under the ACT
bound.  Init (z0@W1) and final (A@W2)
matmuls run in fp16 so the displacement z_final - z0 only carries fp8 noise
through the dynamics, not through the final projection (numpy model of this
scheme: rel err ~7e-3 vs the fp32 reference; gate is 2e-2).

Data parallel over 8 cores (1024 batch rows each), feature-major layout,
batch split into 2 chunks of 512 columns so the per-chunk pipeline
PE(mm) -> ACT(tanh) -> PE(next mm) overlaps across chunks.  Steady state is
ACT-bound at ~3.95us/step (2 tanh ops of [128,2048] @ 1.2GHz); PE is at
~3.44us/step (16 DoubleRow matmuls @ 215ns).

Head: input DMAs for z16/w1s are split per contraction-plane so the init
matmuls start as soon as the first 256KB lands, and the PE warmup bridges
the DMA latency so HAM never re-throttles.  Tail: output DMAs alternate
between two engine queues.
"""

import numpy as np
import ml_dtypes

P = 128
D = 512
B_FULL = 8192
NCORES = 8
BSH = B_FULL // NCORES  # 1024 batch rows per core
NSTEPS = 20
CB = 512                # batch columns per chunk
NCHUNK = BSH // CB      # 2 chunks
FT = D // P             # 4 feature tiles
S = 128.0               # fp32 state scale (keeps fp8 M~ in normal range)
NWARM = 26              # PE prewarm matmuls (HAM clock ramp) during DMA

_CACHE = {}


def _build_nc(has_bias):
    import concourse.bacc as bacc
    import concourse.mybir as mybir
    import concourse.tile as tile
    from concourse.masks import make_identity

    f32 = mybir.dt.float32
    f16 = mybir.dt.float16
    f8 = mybir.dt.float8e4
    Tanh = mybir.ActivationFunctionType.Tanh
    DR = mybir.MatmulPerfMode.DoubleRow

    nc = bacc.Bacc("TRN2", target_bir_lowering=False, debug=False)
    # All inputs arrive pre-tiled in SBUF layout (4KB contiguous rows) so
    # each tensor is one large DMA at near-peak HBM bandwidth.  The init-
    # critical pair (z0 chunk-0 + S*W1) ships as two 512KB halves so the
    # first half unblocks init waves kt=0,1 ~2us earlier.
    # head_a planes: [z0k0, z0k1, w1k0, w1k1]; head_b: [z0k2, z0k3, w1k2, w1k3]
    ha_in = nc.dram_tensor("head_a", [P, FT, D], f16, kind="ExternalInput")
    hb_in = nc.dram_tensor("head_b", [P, FT, D], f16, kind="ExternalInput")
    z1_in = nc.dram_tensor("z16c1", [P, FT, CB], f16, kind="ExternalInput")
    w2h_in = nc.dram_tensor("w2h", [P, FT, D], f16, kind="ExternalInput")  # h*W2
    mq_in = nc.dram_tensor("mq", [P, FT, D], f8, kind="ExternalInput")  # S*h*W2@W1
    if has_bias:
        # biases[p, jt, s] = b1[jt*128+p] + s * (b2 @ W1 * h)[jt*128+p]
        b_in = nc.dram_tensor("biases", [P, FT, NSTEPS], f32, kind="ExternalInput")
        # bfin_row[0, j] = NSTEPS * h * b2[j]
        bf_in = nc.dram_tensor("bfin", [1, D], f16, kind="ExternalInput")
    z_out = nc.dram_tensor("zout", [D, BSH], f16, kind="ExternalOutput")

    zout_t = z_out.ap().rearrange("(ft p) b -> p ft b", p=P)

    def cs(c):
        return slice(c * CB, (c + 1) * CB)

    with tile.TileContext(nc) as tc:
        with (
            tc.tile_pool(name="wpool", bufs=1) as wpool,
            tc.tile_pool(name="a8pool", bufs=4) as a8pool,
            tc.tile_pool(name="ppool", bufs=3) as ppool,
            tc.tile_pool(name="apool", bufs=2) as apool,
            tc.tile_pool(name="opool", bufs=1) as opool,
            tc.tile_pool(name="ps", bufs=1, space="PSUM") as ps,
        ):
            # persistent PSUM state: x~ = S * (z_s @ W1), one 4-bank tile per
            # chunk; bank (c, jt) = xps[c][:, jt, :]
            xps = [ps.tile([P, FT, CB], f32, tag=f"x{c}", name=f"x{c}")
                   for c in range(NCHUNK)]

            # ---- identity + ACT tanh-table preload ----
            ident = wpool.tile([P, P], f32, tag="id")
            make_identity(nc, ident[:])
            warm_sink = wpool.tile([P, P], f32, tag="warm")
            nc.scalar.activation(
                warm_sink[0:1, 0:1], ident[0:1, 0:1], Tanh, scale=1.0,
            )
            ident16 = wpool.tile([P, P], f16, tag="id16")
            nc.vector.tensor_copy(ident16[:], ident[:])

            ha_sb = wpool.tile([P, FT, D], f16, tag="head_a")
            hb_sb = wpool.tile([P, FT, D], f16, tag="head_b")
            nc.sync.dma_start(ha_sb[:], ha_in.ap())
            nc.sync.dma_start(hb_sb[:], hb_in.ap())
            z1_sb = wpool.tile([P, FT, CB], f16, tag="z16_1", name="z16_1")
            mq_sb = wpool.tile([P, FT, D], f8, tag="mq")
            nc.sync.dma_start(z1_sb[:], z1_in.ap())
            nc.sync.dma_start(mq_sb[:], mq_in.ap())

            def z0sl(c, kt):
                # z0 chunk c, feature plane kt
                if c == 1:
                    return z1_sb[:, kt, :]
                return ha_sb[:, kt, :] if kt < 2 else hb_sb[:, kt - 2, :]

            def w1sl(kt, jt):
                sl = slice(jt * P, (jt + 1) * P)
                return (ha_sb[:, 2 + kt, sl] if kt < 2
                        else hb_sb[:, kt, sl])
            if has_bias:
                bias_sb = wpool.tile([P, FT, NSTEPS], f32, tag="bias")
                nc.sync.dma_start(bias_sb[:], b_in.ap())

            # ---- PE prewarm with junk fp16 matmuls into the x banks; the
            # init matmuls below overwrite with start=True.  Bridges the
            # ~8us z16 DMA latency so HAM stays at full clock. ----
            for i in range(NWARM):
                nc.tensor.matmul(
                    xps[i % 2][:, (i // 2) % FT, 0:P], ident16[:], ident16[:],
                    start=True, stop=True, skip_group_check=True,
                )
            nc.vector.tensor_copy(
                warm_sink[:], xps[(NWARM - 1) % 2][:, ((NWARM - 1) // 2) % FT, 0:P])

            # ---- init: x~_0 = z0 @ (S*W1), fp16, kt-outer so each 256KB
            # DMA piece unblocks 8 matmuls ----
            for c in range(NCHUNK):
                for kt in range(FT):
                    for jt in range(FT):
                        nc.tensor.matmul(
                            xps[c][:, jt, :],
                            w1sl(kt, jt),
                            z0sl(c, kt),
                            start=(kt == 0), stop=(kt == FT - 1),
                            skip_group_check=True,
                        )

            # needed only at the final projection; issued after the critical
            # head pieces
            w2h_sb = wpool.tile([P, FT, D], f16, tag="w2h")
            nc.sync.dma_start(w2h_sb[:], w2h_in.ap())
            if has_bias:
                bfin_sb = wpool.tile([1, D], f16, tag="bfin")
                nc.sync.dma_start(bfin_sb[:], bf_in.ap())
                ones_sb = wpool.tile([1, CB], f16, tag="ones")
                nc.vector.memset(ones_sb[:], 1.0)

            # ---- 20 steps: (x~ += a8 @ M~ for s>0), a8 = tanh(x~/S);
            # DVE accumulates A in fp16 from step-pair sums.
            a8 = [None] * NCHUNK       # current a8
            a8p = [None] * NCHUNK      # previous step's a8 (for pair sums)
            acc = [None] * NCHUNK      # fp16 running sum of pairs
            accp = [[None] * FT, [None] * FT]  # final per-plane A16 tiles
            for s in range(NSTEPS):
                for c in range(NCHUNK):
                    if s > 0:
                        for jt in range(FT):
                            for t in range(2):
                                nc.tensor.matmul(
                                    xps[c][:, jt, :],
                                    mq_sb[:, 2 * t:2 * t + 2, jt * P:(jt + 1) * P],
                                    a8[c][:, 2 * t:2 * t + 2, :],
                                    start=False, stop=(t == 1),
                                    perf_mode=DR,
                                    skip_group_check=True,
                                )
                    a8n = a8pool.tile([P, FT, CB], f8, tag=f"a8_{c}", name=f"a8_{c}_{s}")
                    if has_bias:
                        for jt in range(FT):
                            nc.scalar.activation(
                                a8n[:, jt, :], xps[c][:, jt, :], Tanh,
                                bias=bias_sb[:, jt, s:s + 1], scale=1.0 / S,
                            )
                    else:
                        nc.scalar.activation(
                            a8n[:], xps[c][:], Tanh, scale=1.0 / S,
                        )
                    a8p[c] = a8[c]
                    a8[c] = a8n
                    if s == NSTEPS - 1:
                        # last pair, whole tile (fewer DVE ops = fewer sem
                        # hops); shared bufs=1 buffer forces the DVE order
                        # p(c0) -> A16(c0) -> p(c1) -> A16(c1) so chunk 0's
                        # final matmuls unblock as early as possible
                        pl = ppool.tile([P, FT, CB], f16, tag="p_last",
                                        bufs=1, name=f"pl_{c}")
                        nc.vector.tensor_add(pl[:], a8p[c][:], a8n[:])
                        an = apool.tile([P, FT, CB], f16, tag=f"A16_{c}",
                                        name=f"A16_{c}")
                        nc.vector.tensor_add(an[:], acc[c][:], pl[:])
                        acc[c] = an
                    elif s % 2 == 1:
                        # pair sum p = a_{s-1} + a_s (fp8+fp8 -> fp16, 1x)
                        pn = ppool.tile([P, FT, CB], f16, tag=f"p_{c}",
                                        name=f"p_{c}_{s}")
                        nc.vector.tensor_add(pn[:], a8p[c][:], a8n[:])
                        if s == 1:
                            acc[c] = pn
                        else:
                            # A += p, all-fp16 (2x_1P)
                            an = apool.tile([P, FT, CB], f16, tag=f"A_{c}",
                                            name=f"A_{c}_{s}")
                            nc.vector.tensor_add(an[:], acc[c][:], pn[:])
                            acc[c] = an

            # ---- junk matmuls to keep HAM warm while the last A16 adds run
            # (chunk-0 banks only: chunk-1 targets would stall the PE queue
            # on the last tanh)
            for i in range(20):
                nc.tensor.matmul(
                    xps[0][:, i % FT, 0:P], ident16[:], ident16[:],
                    start=True, stop=True, skip_group_check=True,
                )

            # ---- final: z = z0 + (A fp16) @ (h*W2) (+ NSTEPS*h*b2) ----
            # All matmuls first, then all zo adds: interleaving them chains
            # whole-tile WAR deps (zo reads xps) and serializes the phase.
            for c in range(NCHUNK):
                for kt in range(FT):
                    for jt2 in range(FT):
                        nc.tensor.matmul(
                            xps[c][:, jt2, :],
                            w2h_sb[:, kt, jt2 * P:(jt2 + 1) * P],
                            acc[c][:, kt, :],
                            start=(kt == 0),
                            stop=(kt == FT - 1) and not has_bias,
                            skip_group_check=True,
                        )
                if has_bias:
                    for jt2 in range(FT):
                        nc.tensor.matmul(
                            xps[c][:, jt2, :],
                            bfin_sb[:, jt2 * P:(jt2 + 1) * P],
                            ones_sb[:],
                            start=False, stop=True,
                            skip_group_check=True,
                        )
                # z0 is re-read from the resident fp16 init input; output
                # ships fp16 (host casts back).  Chunk 1's four adds merge
                # into one DVE op (z1_sb is one contiguous tile) — its zo
                # chain is the exposed end of the kernel.
                if c == 1:
                    zo1 = opool.tile([P, FT, CB], f16, tag="zo_1", name="zo_1")
                    nc.vector.tensor_add(zo1[:], xps[1][:], z1_sb[:])
                    for jt2 in range(FT):
                        eng = nc.sync if jt2 % 2 == 0 else nc.scalar
                        eng.dma_start(zout_t[:, jt2, cs(1)], zo1[:, jt2, :])
                else:
                    for jt2 in range(FT):
                        zo = opool.tile([P, CB], f16, tag=f"zo_0_{jt2}",
                                        name=f"zo_0_{jt2}")
                        nc.vector.tensor_add(
                            zo[:], xps[0][:, jt2, :], z0sl(0, jt2),
                        )
                        eng = nc.sync if jt2 % 2 == 0 else nc.scalar
                        eng.dma_start(zout_t[:, jt2, cs(0)], zo[:])

    nc.finalize()
    return nc


def _get_nc(has_bias):
    key = ("nc", has_bias)
    if key not in _CACHE:
        _CACHE[key] = _build_nc(has_bias)
    return _CACHE[key]


def _prepare_inputs(z0, t, W1, b1, W2, b2):
    z0 = np.asarray(z0, dtype=np.float32)
    t = np.asarray(t, dtype=np.float32)
    W1 = np.asarray(W1, dtype=np.float32)
    b1 = np.asarray(b1, dtype=np.float64)
    W2 = np.asarray(W2, dtype=np.float32)
    b2 = np.asarray(b2, dtype=np.float64)

    h = (float(t[1]) - float(t[0])) / NSTEPS
    has_bias = bool(np.any(b1 != 0.0) or np.any(b2 != 0.0))

    zT16 = np.ascontiguousarray(z0.T).astype(np.float16)  # [D, B_FULL]
    # pre-tile to SBUF layout [p, plane, col] (plane q holds rows 128q+p):
    # each DMA then moves 4KB contiguous rows at near-peak HBM bandwidth
    zt = zT16.reshape(FT, P, B_FULL).transpose(1, 0, 2)  # [P, FT, B_FULL]

    def tile_w(w):  # [D, D] -> [P, FT, D]
        return np.ascontiguousarray(w.reshape(FT, P, D).transpose(1, 0, 2))

    w1s = tile_w((W1.astype(np.float64) * S).astype(np.float32).astype(np.float16))
    w2h = tile_w((W2.astype(np.float64) * h).astype(np.float32).astype(np.float16))
    M = (W2.astype(np.float64) @ W1.astype(np.float64)) * (h * S)
    mq = tile_w(M.astype(np.float32).astype(ml_dtypes.float8_e4m3))

    extras = {}
    if has_bias:
        wtb = (b2 @ W1.astype(np.float64)) * h      # [D]
        biases = np.stack(
            [b1 + s * wtb for s in range(NSTEPS)], axis=0
        ).astype(np.float32)                         # [NSTEPS, D]
        extras["biases"] = np.ascontiguousarray(
            biases.reshape(NSTEPS, FT, P).transpose(2, 1, 0))
        extras["bfin"] = np.ascontiguousarray(
            (NSTEPS * h * b2).astype(np.float32).astype(np.float16).reshape(1, D))

    in_maps = []
    for i in range(NCORES):
        z0c0 = zt[:, :, i * BSH:i * BSH + CB]          # [P, FT, CB]
        z0c1 = zt[:, :, i * BSH + CB:(i + 1) * BSH]
        m = {
            "head_a": np.ascontiguousarray(
                np.concatenate([z0c0[:, 0:2], w1s[:, 0:2]], axis=1)),
            "head_b": np.ascontiguousarray(
                np.concatenate([z0c0[:, 2:4], w1s[:, 2:4]], axis=1)),
            "z16c1": np.ascontiguousarray(z0c1),
            "w2h": w2h,
            "mq": mq,
        }
        m.update(extras)
        in_maps.append(m)
    return in_maps, has_bias


def _run(in_maps, has_bias, trace=False):
    from concourse import bass_utils

    nc = _get_nc(has_bias)
    res = bass_utils.run_bass_kernel_spmd(
        nc, in_maps, core_ids=list(range(NCORES)), trace=trace,
    )
    return res


def kernel(z0, t, W1, b1, W2, b2):
    in_maps, has_bias = _prepare_inputs(z0, t, W1, b1, W2, b2)
    res = _run(in_maps, has_bias)
    outT = np.concatenate([r["zout"] for r in res.results], axis=1)  # [D, B] f16
    return np.ascontiguousarray(outT.T).astype(np.float32)
